# revision 46
# baseline (speedup 1.0000x reference)
"""Multi-head attention (B=2, S=2048, D=1024, H=16) on 8 Trainium2 cores.

Sharding: data-parallel over batch (2) x tensor-parallel over heads (16 -> 4
per core). Core c handles batch c//4, heads 4*(c%4) .. 4*(c%4)+3. Each core
computes its heads' Q/K/V projections (column-sliced weights), flash-style
attention, and a partial output projection (row-sliced Wo). Host sums the 4
partials per batch and adds bv@Wo + bo.

v7 design (226-230us, from 270us v5):
  - x arrives host-pre-transposed [D, S] bf16: 32 contiguous (chunk x
    seq-quarter) DMAs land x^T in SBUF with no PE/DVE transpose work.
  - Block-0 q/k projections are emitted first (chunk-interleaved so their
    d-chunk matmuls pipeline against the x/wk/wq DMA trickle); the exp
    stream starts ~21us into the kernel.
  - Attention unit u = (i-block of 512 queries, head-pair); j-loop over 16
    key tiles: scores [128 keys, 2 heads x 512 q] via row-packed K=64 pairs
    (the two tile_position matmuls stream CONCURRENTLY, ~215ns/pair) ->
    exp on ACT [128, 1024] (1.03us, the cadence-setter) -> orientation-C AV.
  - Orientation-C AV: out[q, d] += ex_slice[128k, 128q]^T @ va[128k, 65].
    8 small matmuls per j (FWL weight loads overlap the 65-col streams;
    ~260ns wall vs 430ns for the [65, 1024] orientation). The va ones-column
    accumulates the softmax denominator per query ON the q-partition, so
    normalization is a per-partition reciprocal [128, 8] + 8 tensor_scalar
    multiplies (DVE) instead of rank-1 broadcast matmuls + a 3.6us
    reciprocal. PE start=True wipes the accumulation state of the whole
    PSUM bank, so the 8 sub-block groups are pre-zeroed by one DVE memset
    and accumulate with start=False + skip_group_check.
  - Normalized [128 q, 64] blocks transpose back to the Wo-lhsT layout via
    tiny PE transposes (identity matmuls) hosted in later j-steps.
  - All other work (v-proj, remaining q/k projections split into 4-chunk
    halves, out-projection row-tiles) drains from a dependency-tracked
    queue, budget-paced against the exp cadence: each exp's semaphore wait
    covers ALL prior PE work, so any burst of background PE work between
    two exps stalls the stream for its full duration.
  - PSUM: short [128,1024] x2 (scores ring), small [128,512] x2 (proj/fin/
    transposes), long [128, 2, 4, 128] x1 (AV accumulator; 512B-aligned
    sub-blocks so no group crosses a 2KB bank).
  - Output DMAs ride the sync queue (a DMA instruction occupies its HWDGE
    queue's sequencer ~600ns, so the scalar queue carries them only at the
    tail when the exp stream is done).
"""

import numpy as np

B, S, D, H, DK = 2, 2048, 1024, 16, 64
HPC = 4          # heads per core
HD = HPC * DK    # 256 projected dims per core
P = 128
NB = 512
NCORES = 8

_CACHE = {}


def _install_tile_drain_fix():
    """TileContext._drain_and_barrier piles every outstanding sem wait onto
    one Drain instruction; this walrus build rejects >1 sync wait per
    instruction. Split the extra waits across single-wait NOPs."""
    import concourse.tile as tile
    from concourse.vector_clock import ScopedClock

    if getattr(tile.TileContext, "_ant_drain_fix", False):
        return

    def _drain_and_barrier_split(self, tick_clock, wait_clock):
        drain_inst = self.nc.sync.drain()
        wait_clock.add_sem_waits(
            drain_inst.ins, ScopedClock({None: tick_clock.global_clock})
        )
        waits = list(drain_inst.ins.sync_info.on_wait or [])
        if len(waits) > 1:
            drain_inst.ins.sync_info.on_wait = waits[:1]
            for w in waits[1:]:
                n = self.nc.sync.nop(nofuse=True)
                si = n.ins.sync_info
                if si is None:
                    import bass_rust

                    n.ins.sync_info = bass_rust.SyncInfo(on_wait=[w], on_update=[])
                else:
                    si.on_wait = [w]

        self.nc.all_engine_barrier()
        assert self.sems is not None
        popped = self.nc._tile_sem_poison_stack.pop()
        assert popped is self._sem_poison
        self.nc.clear_and_free_semaphores(list(self.sems.allocated().values()))
        self.nc.all_engine_barrier()

    tile.TileContext._drain_and_barrier = _drain_and_barrier_split
    tile.TileContext._ant_drain_fix = True


def _split_excess_waits(nc):
    """walrus's per-struct sync-wait capacity is small (observed: 1 for the
    self-loading-weight Matmult S3_LW struct, 2 for TPB_CTRL/Drain). Tile's
    wait assignment can leave many waits on one instruction; hoist the excess
    onto NOPs on the same engine immediately before it."""
    import concourse.mybir as mybir

    nid = [0]
    for f in nc.m.functions:
        for bb in f.blocks:
            out = []
            changed = False
            for inst in bb.instructions:
                si = getattr(inst, "sync_info", None)
                waits = list(si.on_wait) if si is not None and si.on_wait else []
                cap = 1
                if len(waits) > cap:
                    extra = waits[cap:]
                    for k in range(0, len(extra), 2):
                        nid[0] += 1
                        out.append(
                            mybir.InstEventSemaphore(
                                name=f"I-waitsplit-{nid[0]}",
                                ins=[],
                                outs=[],
                                sync_info=mybir.SyncInfo(
                                    on_wait=extra[k:k + 2], on_update=[]
                                ),
                                engine=inst.engine,
                            )
                        )
                    si.on_wait = waits[:cap]
                    changed = True
                out.append(inst)
            if changed:
                bb.instructions = out


_MM_TAGS = []   # analysis aid: phase tag per emitted PE matmul, program order
_CUR_TAG = ["?"]


def _build_program():
    import concourse.bass as bass
    import concourse.mybir as mybir
    from concourse.tile import TileContext

    _install_tile_drain_fix()

    del _MM_TAGS[:]

    f32 = mybir.dt.float32
    f32r = mybir.dt.float32r
    bf16 = mybir.dt.bfloat16
    Exp = mybir.ActivationFunctionType.Exp

    nc = bass.Bass()

    xbh = nc.dram_tensor("xbh", [D, S], bf16, kind="ExternalInput")
    # weights arrive host-pre-permuted to [p, chunk, h] so their DMAs are
    # fully contiguous (the strided (c p) h -> p c h gather runs at ~16GB/s
    # and blocks the scalar HWDGE queue for ~40us)
    wq = nc.dram_tensor("wq", [P, D // P * HD], bf16, kind="ExternalInput")
    wk = nc.dram_tensor("wk", [P, D // P * HD], bf16, kind="ExternalInput")
    wv = nc.dram_tensor("wv", [P, D // P * HD], bf16, kind="ExternalInput")
    wo = nc.dram_tensor("wo", [P, 2 * D], bf16, kind="ExternalInput")
    bqt = nc.dram_tensor("bqt", [P, 2], f32, kind="ExternalInput")
    bkt = nc.dram_tensor("bkt", [P, 2], f32, kind="ExternalInput")
    outp = nc.dram_tensor("outp", [S, D], f32, kind="ExternalOutput")

    NDC = D // P      # 8 d-chunks
    NST = S // P      # 16 sequence tiles
    NSB = S // NB     # 4 sequence blocks

    _orig_mm = nc.tensor.matmul

    def _tagged_mm(*a, **k):
        _MM_TAGS.append(_CUR_TAG[0])
        return _orig_mm(*a, **k)

    nc.tensor.matmul = _tagged_mm

    from concourse.masks import make_identity

    with TileContext(nc) as tc:
        with (
            tc.tile_pool(name="sb", bufs=1) as consts,
            tc.tile_pool(name="ps", bufs=1, space="PSUM") as ps,
        ):
            wconst = consts.tile([1, 16], f32)
            nc.vector.memset(wconst[:], 1.0)
            warm = consts.tile([1, 16], f32)

            # weights ride the scalar-engine HWDGE queue so the x chunk DMAs
            # (sync queue) aren't serialized behind them. Order: biases (tiny),
            # then wk/wq (needed ~2us in for the first scores), wv, wo.
            identf = consts.tile([P, P], f32)
            make_identity(nc, identf)
            identb = consts.tile([P, P], bf16)
            nc.vector.tensor_copy(out=identb[:], in_=identf[:])
            wk_sb = consts.tile([P, NDC, HD], bf16)
            nc.scalar.dma_start(wk_sb[:], wk.rearrange("p (c h) -> p c h", c=NDC))
            wq_sb = consts.tile([P, NDC, HD], bf16)
            nc.scalar.dma_start(wq_sb[:], wq.rearrange("p (c h) -> p c h", c=NDC))
            bq_sb = consts.tile([P, 16], f32)
            nc.scalar.dma_start(bq_sb[:, 0:2], bqt[:])
            bk_sb = consts.tile([P, 16], f32)
            nc.scalar.dma_start(bk_sb[:, 0:2], bkt[:])
            wv_sb = consts.tile([P, NDC, HD], bf16)
            nc.scalar.dma_start(wv_sb[:], wv.rearrange("p (c h) -> p c h", c=NDC))
            # pre-warm the ACT exp table set (~1.3us): must precede exp(0,0)
            nc.scalar.activation(warm[0:1, 0:1], wconst[0:1, 0:1], Exp)
            # wo's DMA is emitted after the x transposes (it's needed ~90us
            # in, and would delay the scalar-queue transposes otherwise)
            wo_sb = consts.tile([P, 2, D], bf16)

            if True:
                acts = consts
                xT = acts.tile([P, NDC, S], bf16)
                # pair-packed transposed projections: [2 heads x 64, S]
                qT = acts.tile([P, 2, S], bf16)
                kT = acts.tile([P, 2, S], bf16)
                # v augmented with a ones column (row 65 of the AV matmul
                # accumulates the softmax denominator): [s, j-tile, head, 65]
                va = acts.tile([P, NST, HPC, DK + 1], bf16)
                # Wo lhsT: [head-dim pair-chunk, pair, i]
                stack = acts.tile([P, 2, S], bf16)

                nc.scalar.dma_start(
                    wo_sb[:], wo.rearrange("p (c d) -> p c d", c=2)
                )

                def short(nm):
                    return ps.tile([P, 2 * NB], f32, tag="short", bufs=2, name=nm)

                def small(nm):
                    return ps.tile([P, NB], f32, tag="small", bufs=2, name=nm)

                def long_(nm):
                    # orientation-C AV accumulator: [q, head, q-tile, 68]
                    # (cols 0-63 dims, col 64 softmax denominator, 68 pads
                    # the per-block stride to a 16B multiple)
                    return ps.tile([P, 2, NB // P, P], f32, tag="long",
                                   bufs=1, name=nm)

                # ---------------- projections -------------------------------
                def emit_xdma(c, q4):
                    # x arrives host-pre-transposed [D, S]: plain contiguous
                    # 2D DMA per (chunk, seq-quarter) -> 32 cheap descriptors
                    nc.sync.dma_start(
                        xT[:, c, q4 * NB:(q4 + 1) * NB],
                        xbh[c * P:(c + 1) * P, q4 * NB:(q4 + 1) * NB],
                    )

                def emit_vproj(it):
                    _CUR_TAG[0] = f"vp{it}"
                    vp = small(f"vp{it}")
                    for d in range(NDC):
                        nc.tensor.matmul(
                            vp[:, 0:HD],
                            xT[:, d, it * P:(it + 1) * P],
                            wv_sb[:, d, :],
                            start=(d == 0),
                            stop=(d == NDC - 1),
                        )
                    nc.vector.tensor_copy(
                        out=va[:, it, :, 0:DK],
                        in_=vp[:, 0:HD].rearrange("p (h e) -> p h e", h=HPC),
                    )

                def emit_proj_part(sb, p, col, part):
                    # half of a q/k projection (4 d-chunks): small enough to
                    # fit a j-step's PE slack without stalling the exp stream
                    _CUR_TAG[0] = f"pj{sb}_{p}_{col}"
                    w_sb, b_sb, dT = (
                        (wq_sb, bq_sb, qT) if col == 0 else (wk_sb, bk_sb, kT)
                    )
                    if part == 0:
                        pq = small(f"pj{sb}_{p}_{col}")
                        state[("pj", sb, p, col)] = pq
                    else:
                        pq = state.pop(("pj", sb, p, col))
                    for d in range(4 * part, 4 * part + 4):
                        nc.tensor.matmul(
                            pq[:],
                            w_sb[:, d, p * P:(p + 1) * P],
                            xT[:, d, sb * NB:(sb + 1) * NB],
                            start=(d == 0),
                            stop=(d == NDC - 1),
                        )
                    if part == 1:
                        with nc.allow_low_precision("bf16 q/k feed scores"):
                            nc.vector.tensor_scalar_add(
                                out=dT[:, p, sb * NB:(sb + 1) * NB],
                                in0=pq[:],
                                scalar1=b_sb[:, p:p + 1],
                            )

                def emit_proj_half(sb, p, col):
                    emit_proj_part(sb, p, col, 0)
                    emit_proj_part(sb, p, col, 1)

                # ---------------- attention helpers -------------------------
                # all pair-0 units first; projections/finish work are drained
                # from a dependency-tracked background queue into step slots
                units = [(ib, 0) for ib in range(NSB)] + [
                    (ib, 1) for ib in range(NSB)
                ]

                def emit_scores(u, j):
                    _CUR_TAG[0] = f"sc{u}_{j}"
                    ib, p = units[u]
                    i0 = ib * NB
                    sc = short(f"sc{u}_{j}")
                    nc.tensor.matmul(
                        sc[:, 0:NB],
                        kT[0:DK, p, j * P:(j + 1) * P],
                        qT[0:DK, p, i0:i0 + NB],
                        tile_position=(0, 0),
                    )
                    nc.tensor.matmul(
                        sc[:, NB:2 * NB],
                        kT[DK:2 * DK, p, j * P:(j + 1) * P],
                        qT[DK:2 * DK, p, i0:i0 + NB],
                        tile_position=(64, 0),
                    )
                    return sc

                def emit_scores_pair(u, j):
                    _CUR_TAG[0] = f"scp{u}_{j}"
                    # boundary variant on the 1-bank ring: the 2-deep score
                    # ring drains to zero depth at unit boundaries (its slot
                    # frees only when exp(u-1, 14) completes), which stalls
                    # the exp stream ~2us; these bypass that ring
                    ib, p = units[u]
                    i0 = ib * NB
                    pair = []
                    for h in range(2):
                        sch = small(f"sc{u}_{j}_{h}")
                        nc.tensor.matmul(
                            sch[:, 0:NB],
                            kT[h * DK:(h + 1) * DK, p, j * P:(j + 1) * P],
                            qT[h * DK:(h + 1) * DK, p, i0:i0 + NB],
                            tile_position=(h * DK, 0),
                        )
                        pair.append(sch)
                    return tuple(pair)

                def emit_tc(u, g):
                    # transpose two normalized [128 q, 64] blocks of unit u
                    # into the Wo lhsT (stack) via tiny PE transposes
                    _CUR_TAG[0] = f"tc{u}_{g}"
                    ib, p = units[u]
                    i0 = ib * NB
                    ao = state[f"ao{u}"]
                    for h, qt in (divmod(2 * g, NSB), divmod(2 * g + 1, NSB)):
                        trp = small(f"tc{u}_{h}_{qt}")
                        nc.tensor.matmul(
                            trp[0:DK, 0:P], ao[:, h, qt, :], identb[:],
                        )
                        with nc.allow_low_precision("bf16 stack feeds Wo"):
                            nc.vector.tensor_copy(
                                out=stack[
                                    h * DK:(h + 1) * DK, p,
                                    i0 + qt * P:i0 + (qt + 1) * P,
                                ],
                                in_=trp[0:DK, 0:P],
                            )
                    if g == 3:
                        del state[f"ao{u}"]

                def emit_fin(ib, t, tail=False):
                    _CUR_TAG[0] = f"fin{ib}_{t}"
                    # output projection for row-tile t of i-block ib, split
                    # into two D-halves on the 1-bank "small" ring; at the
                    # tail the idle score ring is used for every other half
                    # so two halves overlap
                    it = ib * (NB // P) + t
                    for nbi in range(2):
                        fint = (
                            short(f"fin{it}_{nbi}") if (tail and nbi == 1)
                            else small(f"fin{it}_{nbi}")
                        )
                        fin = fint[:, 0:NB]
                        for pch in range(2):
                            nc.tensor.matmul(
                                fin,
                                stack[:, pch, it * P:(it + 1) * P],
                                wo_sb[:, pch, nbi * NB:(nbi + 1) * NB],
                                start=(pch == 0),
                                stop=(pch == 1),
                            )
                        ot = acts.tile(
                            [P, NB], f32, tag="ot", bufs=3, name=f"ot{it}_{nbi}"
                        )
                        nc.vector.tensor_copy(out=ot[:], in_=fin)
                        # after the exp stream ends the scalar HWDGE queue is
                        # idle; mid-stream it must not carry DMAs (each DMA
                        # occupies the ACT sequencer ~600ns and stalls exp)
                        eng = nc.scalar if (tail and nbi == 1) else nc.sync
                        eng.dma_start(
                            outp[it * P:(it + 1) * P, nbi * NB:(nbi + 1) * NB],
                            ot[:],
                        )

                from collections import deque

                sc_q = deque()
                fin_q = deque()
                state = {"pending": None, "po": None}
                work_q = deque()
                done = set()
                # virtual timeline: drain background PE work only while the
                # PE is ahead of the ACT exp cadence; co-running PE work under
                # the exp stream costs ACT ~20% (SBUF/PSUM port contention),
                # so the queue must neither starve scores nor front-load
                budget = {"pe": 0.0, "act": 0.0, "j": 0}
                COST = {"vp": 950.0, "pj": 1800.0, "pjB": 900.0, "fin": 950.0, "tc": 250.0}

                def drain_one():
                    if not work_q:
                        return False
                    key, fn, args = work_q.popleft()
                    if key[0] == "pj":
                        emit_proj_part(*key[1:], 0)
                        work_q.appendleft((("pjB",) + key[1:],
                                           emit_proj_part, key[1:] + (1,)))
                        budget["pe"] += 900.0
                        return True
                    fn(*args)
                    done.add(key)
                    if key[0] == "pjB":
                        done.add(("pj",) + key[1:])
                    budget["pe"] += COST.get(key[0], 700.0)
                    return True

                def drain_budget():
                    # cap per-step drains: a burst of background work between
                    # two exps stalls the exp stream for its full duration
                    # (each exp's sem wait covers all prior PE work)
                    spent = 0.0
                    while work_q and budget["pe"] + 300.0 < budget["act"]:
                        c = COST.get(work_q[0][0][0], 700.0)
                        if spent > 0.0 and spent + c > 2800.0:
                            break
                        drain_one()
                        spent += c

                def ensure(*keys):
                    need = [k for k in keys if k not in done]
                    while need:
                        assert work_q, f"missing work for {need}"
                        drain_one()
                        need = [k for k in need if k not in done]

                def emit_unit_end(u):
                    # the ao ring is 2 deep: unit u's normalize reuses the
                    # slot of u-2, whose readers are the tc items queued at
                    # our j==3 -- they must be emitted before we are
                    if u >= 1:
                        ensure(*[("tc", u - 1, g) for g in range(4)])
                    # per-query reciprocal of the 8 denominator columns, then
                    # normalize each [128 q, 64] block into bf16 (DVE); the
                    # accumulator bank frees right here, not at the next bc
                    po = state["po"]
                    rcp = acts.tile([P, 16], f32, tag="rcp", bufs=2,
                                    name=f"rcp{u}")
                    with nc.allow_low_precision("approx recip of sumexp"):
                        for h in range(2):
                            for qt in range(NSB):
                                i_ = 4 * h + qt
                                nc.vector.reciprocal(
                                    out=rcp[:, i_:i_ + 1],
                                    in_=po[:, h, qt, 64:65],
                                )
                    ao = acts.tile([P, 2, NSB, DK], bf16, tag="aos", bufs=2,
                                   name=f"ao{u}")
                    with nc.allow_low_precision("bf16 normalized attn out"):
                        for h in range(2):
                            for qt in range(NSB):
                                nc.vector.tensor_scalar_mul(
                                    out=ao[:, h, qt, :],
                                    in0=po[:, h, qt, 0:DK],
                                    scalar1=rcp[:, 4 * h + qt:4 * h + qt + 1],
                                )
                    state[f"ao{u}"] = ao
                    state["pending"] = u

                def emit_step(u, j):
                    # one attention j-step of unit u
                    ib, p = units[u]
                    if j == 0:
                        state["po"] = long_(f"po{u}")
                        # PE start=True wipes the accumulation state of the
                        # whole PSUM bank (not just the written region), so
                        # zero the 8 sub-blocks here and accumulate with
                        # start=False throughout
                        nc.vector.memset(state["po"][:, :, :, 0:65], 0.0)
                    po = state["po"]
                    sc = sc_q.popleft()
                    ex = acts.tile(
                        [P, 2 * NB], bf16, tag="ex", bufs=6, name=f"ex{u}_{j}"
                    )
                    if isinstance(sc, tuple):
                        for h in range(2):
                            nc.scalar.activation(
                                ex[:, h * NB:(h + 1) * NB], sc[h][:, 0:NB],
                                Exp, scale=0.125,
                            )
                    else:
                        nc.scalar.activation(ex[:], sc[:], Exp, scale=0.125)
                    nj = j + 2
                    if nj < NST:
                        ensure(("pj", ib, p, 0), ("pj", nj // 4, p, 1))
                        sc_q.append(emit_scores(u, nj))
                    elif u + 1 < 8:
                        ib2, p2 = units[u + 1]
                        ensure(("pj", ib2, p2, 0), ("pj", 0, p2, 1))
                        if work_q and work_q[0][0][0] == "pjB":
                            drain_one()
                        sc_q.append(emit_scores_pair(u + 1, nj - NST))
                    def emit_av(aj, aex):
                        _CUR_TAG[0] = f"av{u}_{aj}"
                        ensure(("vp", aj))
                        for h in range(2):
                            for qt in range(NSB):
                                nc.tensor.matmul(
                                    po[:, h, qt, 0:DK + 1],
                                    aex[:, h * NB + qt * P:
                                        h * NB + (qt + 1) * P],
                                    va[:, aj, 2 * p + h, :],
                                    start=False,
                                    stop=(aj == NST - 1),
                                    skip_group_check=True,
                                )

                    # defer the first two AVs to step 2: the po-ring WAR
                    # (drains of the previous unit on DVE) then never heads
                    # the in-order PE queue at unit boundaries
                    if j in (0, 1):
                        state.setdefault("av_defer", []).append((j, ex))
                    else:
                        for a in state.pop("av_defer", []):
                            emit_av(*a)
                        emit_av(j, ex)
                    # previous-unit finish + budget-paced background work
                    budget["act"] += 1300.0
                    budget["pe"] += 680.0
                    if state["pending"] is not None and j == 3:
                        pu = state["pending"]
                        for g in range(4):
                            work_q.append((("tc", pu, g), emit_tc, (pu, g)))
                        if units[pu][1] == 1:
                            for t in range(NB // P):
                                work_q.append((
                                    ("fin", units[pu][0], t),
                                    emit_fin, (units[pu][0], t),
                                ))
                        state["pending"] = None
                    drain_budget()
                    if j == NST - 1:
                        emit_unit_end(u)

                # ---------------- fused emission ----------------------------
                # x chunk DMAs up front (quarter-major so the first seq block
                # lands ~2.8us in); then block-0 q/k projections + first two
                # score tiles, so the exp stream starts ~5us into the kernel.
                # Everything else drains from work_q one slot per j-step.
                for q4 in range(NSB):
                    for c in range(NDC):
                        emit_xdma(c, q4)
                nc.vector.memset(
                    va[:, :, :, DK:DK + 1].bitcast(mybir.dt.uint16), 0x3F80
                )
                _CUR_TAG[0] = "pj0_0_x"
                pqk = small("pj0_0_1")
                pqq = small("pj0_0_0")
                for d in range(NDC):
                    nc.tensor.matmul(
                        pqk[:], wk_sb[:, d, 0:P], xT[:, d, 0:NB],
                        start=(d == 0), stop=(d == NDC - 1),
                    )
                    nc.tensor.matmul(
                        pqq[:], wq_sb[:, d, 0:P], xT[:, d, 0:NB],
                        start=(d == 0), stop=(d == NDC - 1),
                    )
                with nc.allow_low_precision("bf16 q/k feed scores"):
                    nc.vector.tensor_scalar_add(
                        out=kT[:, 0, 0:NB], in0=pqk[:], scalar1=bk_sb[:, 0:1]
                    )
                    nc.vector.tensor_scalar_add(
                        out=qT[:, 0, 0:NB], in0=pqq[:], scalar1=bq_sb[:, 0:1]
                    )
                done.add(("pj", 0, 0, 1))
                done.add(("pj", 0, 0, 0))
                sc_q.append(emit_scores(0, 0))
                sc_q.append(emit_scores(0, 1))
                wk_order = (
                    [("vp", 0), ("pj", 1, 0, 1), ("vp", 1), ("pj", 2, 0, 1),
                     ("vp", 2), ("pj", 3, 0, 1)]
                    + [("vp", it) for it in range(3, NST)]
                    + [("pj", sb, 0, 0) for sb in range(1, NSB)]
                    + [("pj", sb, 1, c) for sb in range(NSB) for c in (1, 0)]
                )
                for key in wk_order:
                    if key[0] == "vp":
                        work_q.append((key, emit_vproj, key[1:]))
                    else:
                        work_q.append((key, emit_proj_half, key[1:]))
                for u in range(8):
                    for j in range(NST):
                        emit_step(u, j)
                # tail: finish of the last unit + any remaining queued work
                while work_q:
                    drain_one()
                pu = state["pending"]
                for g in range(4):
                    emit_tc(pu, g)
                for t in range(NB // P):
                    emit_fin(units[pu][0], t, tail=True)

    _split_excess_waits(nc)
    return nc


def _get_program():
    if "nc" not in _CACHE:
        _CACHE["nc"] = _build_program()
    return _CACHE["nc"]


def _prep_w(w):
    # [n*128, m] -> [128, n*m]: partition p holds rows {c*128+p}, contiguous
    n = w.shape[0] // P
    return np.ascontiguousarray(
        w.reshape(n, P, w.shape[1]).transpose(1, 0, 2).reshape(P, -1)
    )


def kernel(x, Wq, bq, Wk, bk, Wv, bv, Wo, bo, _trace=False):
    import ml_dtypes
    from concourse.bass_utils import run_bass_kernel_spmd

    bft = np.dtype(ml_dtypes.bfloat16)
    x = np.asarray(x, dtype=np.float32)
    Wq = np.asarray(Wq, dtype=np.float32)
    Wk = np.asarray(Wk, dtype=np.float32)
    Wv = np.asarray(Wv, dtype=np.float32)
    Wo = np.asarray(Wo, dtype=np.float32)
    bq = np.asarray(bq, dtype=np.float32)
    bk = np.asarray(bk, dtype=np.float32)
    bv = np.asarray(bv, dtype=np.float32)
    bo = np.asarray(bo, dtype=np.float32)

    in_maps = []
    for c in range(NCORES):
        b = c // 4
        cs = (c % 4) * HD
        in_maps.append({
            "xbh": np.ascontiguousarray(x[b].astype(bft).T),
            "wq": _prep_w(Wq[:, cs:cs + HD].astype(bft)),
            "wk": _prep_w(Wk[:, cs:cs + HD].astype(bft)),
            "wv": _prep_w(Wv[:, cs:cs + HD].astype(bft)),
            "wo": _prep_w(Wo[cs:cs + HD, :].astype(bft)),
            "bqt": np.ascontiguousarray(bq[cs:cs + HD].reshape(2, P).T),
            "bkt": np.ascontiguousarray(bk[cs:cs + HD].reshape(2, P).T),
        })

    nc = _get_program()
    res = run_bass_kernel_spmd(
        nc, in_maps, core_ids=list(range(NCORES)), trace=_trace
    )

    cvec = (bv @ Wo + bo).astype(np.float32)
    out = np.empty((B, S, D), dtype=np.float32)
    for b in range(B):
        acc = res.results[4 * b]["outp"].astype(np.float64)
        for c in range(4 * b + 1, 4 * b + 4):
            acc = acc + res.results[c]["outp"]
        out[b] = (acc + cvec).astype(np.float32)

    if _trace:
        _CACHE["last_results"] = res
    return out



# revision 47
# speedup vs baseline: 1.0659x; 1.0659x over previous
"""Multi-head attention (B=2, S=2048, D=1024, H=16) on 8 Trainium2 cores.

Sharding: data-parallel over batch (2) x tensor-parallel over heads (16 -> 4
per core). Core c handles batch c//4, heads 4*(c%4) .. 4*(c%4)+3. Each core
computes its heads' Q/K/V projections (column-sliced weights), flash-style
attention, and a partial output projection (row-sliced Wo). Host sums the 4
partials per batch and adds bv@Wo + bo.

v7 design (226-230us, from 270us v5):
  - x arrives host-pre-transposed [D, S] bf16: 32 contiguous (chunk x
    seq-quarter) DMAs land x^T in SBUF with no PE/DVE transpose work.
  - Block-0 q/k projections are emitted first (chunk-interleaved so their
    d-chunk matmuls pipeline against the x/wk/wq DMA trickle); the exp
    stream starts ~21us into the kernel.
  - Attention unit u = (i-block of 512 queries, head-pair); j-loop over 16
    key tiles: scores [128 keys, 2 heads x 512 q] via row-packed K=64 pairs
    (the two tile_position matmuls stream CONCURRENTLY, ~215ns/pair) ->
    exp on ACT [128, 1024] (1.03us, the cadence-setter) -> orientation-C AV.
  - Orientation-C AV: out[q, d] += ex_slice[128k, 128q]^T @ va[128k, 65].
    8 small matmuls per j (FWL weight loads overlap the 65-col streams;
    ~260ns wall vs 430ns for the [65, 1024] orientation). The va ones-column
    accumulates the softmax denominator per query ON the q-partition, so
    normalization is a per-partition reciprocal [128, 8] + 8 tensor_scalar
    multiplies (DVE) instead of rank-1 broadcast matmuls + a 3.6us
    reciprocal. PE start=True wipes the accumulation state of the whole
    PSUM bank, so the 8 sub-block groups are pre-zeroed by one DVE memset
    and accumulate with start=False + skip_group_check.
  - Normalized [128 q, 64] blocks transpose back to the Wo-lhsT layout via
    tiny PE transposes (identity matmuls) hosted in later j-steps.
  - All other work (v-proj, remaining q/k projections split into 4-chunk
    halves, out-projection row-tiles) drains from a dependency-tracked
    queue, budget-paced against the exp cadence: each exp's semaphore wait
    covers ALL prior PE work, so any burst of background PE work between
    two exps stalls the stream for its full duration.
  - PSUM: short [128,1024] x2 (scores ring), small [128,512] x2 (proj/fin/
    transposes), long [128, 2, 4, 128] x1 (AV accumulator; 512B-aligned
    sub-blocks so no group crosses a 2KB bank).
  - Output DMAs ride the sync queue (a DMA instruction occupies its HWDGE
    queue's sequencer ~600ns, so the scalar queue carries them only at the
    tail when the exp stream is done).
"""

import numpy as np

B, S, D, H, DK = 2, 2048, 1024, 16, 64
HPC = 4          # heads per core
HD = HPC * DK    # 256 projected dims per core
P = 128
NB = 512
NCORES = 8

_CACHE = {}


def _install_tile_drain_fix():
    """TileContext._drain_and_barrier piles every outstanding sem wait onto
    one Drain instruction; this walrus build rejects >1 sync wait per
    instruction. Split the extra waits across single-wait NOPs."""
    import concourse.tile as tile
    from concourse.vector_clock import ScopedClock

    if getattr(tile.TileContext, "_ant_drain_fix", False):
        return

    def _drain_and_barrier_split(self, tick_clock, wait_clock):
        drain_inst = self.nc.sync.drain()
        wait_clock.add_sem_waits(
            drain_inst.ins, ScopedClock({None: tick_clock.global_clock})
        )
        waits = list(drain_inst.ins.sync_info.on_wait or [])
        if len(waits) > 1:
            drain_inst.ins.sync_info.on_wait = waits[:1]
            for w in waits[1:]:
                n = self.nc.sync.nop(nofuse=True)
                si = n.ins.sync_info
                if si is None:
                    import bass_rust

                    n.ins.sync_info = bass_rust.SyncInfo(on_wait=[w], on_update=[])
                else:
                    si.on_wait = [w]

        self.nc.all_engine_barrier()
        assert self.sems is not None
        popped = self.nc._tile_sem_poison_stack.pop()
        assert popped is self._sem_poison
        self.nc.clear_and_free_semaphores(list(self.sems.allocated().values()))
        self.nc.all_engine_barrier()

    tile.TileContext._drain_and_barrier = _drain_and_barrier_split
    tile.TileContext._ant_drain_fix = True


def _split_excess_waits(nc):
    """walrus's per-struct sync-wait capacity is small (observed: 1 for the
    self-loading-weight Matmult S3_LW struct, 2 for TPB_CTRL/Drain). Tile's
    wait assignment can leave many waits on one instruction; hoist the excess
    onto NOPs on the same engine immediately before it."""
    import concourse.mybir as mybir

    nid = [0]
    for f in nc.m.functions:
        for bb in f.blocks:
            out = []
            changed = False
            for inst in bb.instructions:
                si = getattr(inst, "sync_info", None)
                waits = list(si.on_wait) if si is not None and si.on_wait else []
                cap = 1
                if len(waits) > cap:
                    extra = waits[cap:]
                    for k in range(0, len(extra), 2):
                        nid[0] += 1
                        out.append(
                            mybir.InstEventSemaphore(
                                name=f"I-waitsplit-{nid[0]}",
                                ins=[],
                                outs=[],
                                sync_info=mybir.SyncInfo(
                                    on_wait=extra[k:k + 2], on_update=[]
                                ),
                                engine=inst.engine,
                            )
                        )
                    si.on_wait = waits[:cap]
                    changed = True
                out.append(inst)
            if changed:
                bb.instructions = out


_MM_TAGS = []   # analysis aid: phase tag per emitted PE matmul, program order
_CUR_TAG = ["?"]


def _build_program():
    import concourse.bass as bass
    import concourse.mybir as mybir
    from concourse.tile import TileContext

    _install_tile_drain_fix()

    del _MM_TAGS[:]

    f32 = mybir.dt.float32
    f32r = mybir.dt.float32r
    bf16 = mybir.dt.bfloat16
    Exp = mybir.ActivationFunctionType.Exp

    nc = bass.Bass()

    xbh = nc.dram_tensor("xbh", [D, S], bf16, kind="ExternalInput")
    # weights arrive host-pre-permuted to [p, chunk, h] so their DMAs are
    # fully contiguous (the strided (c p) h -> p c h gather runs at ~16GB/s
    # and blocks the scalar HWDGE queue for ~40us)
    wq = nc.dram_tensor("wq", [P, D // P * HD], bf16, kind="ExternalInput")
    wk = nc.dram_tensor("wk", [P, D // P * HD], bf16, kind="ExternalInput")
    wv = nc.dram_tensor("wv", [P, D // P * HD], bf16, kind="ExternalInput")
    wo = nc.dram_tensor("wo", [P, 2 * D], bf16, kind="ExternalInput")
    bqt = nc.dram_tensor("bqt", [P, 2], f32, kind="ExternalInput")
    bkt = nc.dram_tensor("bkt", [P, 2], f32, kind="ExternalInput")
    outp = nc.dram_tensor("outp", [S, D], f32, kind="ExternalOutput")

    NDC = D // P      # 8 d-chunks
    NST = S // P      # 16 sequence tiles
    NSB = S // NB     # 4 sequence blocks

    _orig_mm = nc.tensor.matmul

    def _tagged_mm(*a, **k):
        _MM_TAGS.append(_CUR_TAG[0])
        return _orig_mm(*a, **k)

    nc.tensor.matmul = _tagged_mm

    from concourse.masks import make_identity

    with TileContext(nc) as tc:
        with (
            tc.tile_pool(name="sb", bufs=1) as consts,
            tc.tile_pool(name="ps", bufs=1, space="PSUM") as ps,
        ):
            wconst = consts.tile([1, 16], f32)
            nc.vector.memset(wconst[:], 1.0)
            warm = consts.tile([1, 16], f32)

            # weights ride the scalar-engine HWDGE queue so the x chunk DMAs
            # (sync queue) aren't serialized behind them. Order: biases (tiny),
            # then wk/wq (needed ~2us in for the first scores), wv, wo.
            identf = consts.tile([P, P], f32)
            make_identity(nc, identf)
            identb = consts.tile([P, P], bf16)
            nc.vector.tensor_copy(out=identb[:], in_=identf[:])
            wk_sb = consts.tile([P, NDC, HD], bf16)
            nc.scalar.dma_start(wk_sb[:], wk.rearrange("p (c h) -> p c h", c=NDC))
            wq_sb = consts.tile([P, NDC, HD], bf16)
            nc.scalar.dma_start(wq_sb[:], wq.rearrange("p (c h) -> p c h", c=NDC))
            bq_sb = consts.tile([P, 16], f32)
            nc.scalar.dma_start(bq_sb[:, 0:2], bqt[:])
            bk_sb = consts.tile([P, 16], f32)
            nc.scalar.dma_start(bk_sb[:, 0:2], bkt[:])
            wv_sb = consts.tile([P, NDC, HD], bf16)
            nc.scalar.dma_start(wv_sb[:], wv.rearrange("p (c h) -> p c h", c=NDC))
            # pre-warm the ACT exp table set (~1.3us): must precede exp(0,0)
            nc.scalar.activation(warm[0:1, 0:1], wconst[0:1, 0:1], Exp)
            # wo's DMA is emitted after the x transposes (it's needed ~90us
            # in, and would delay the scalar-queue transposes otherwise)
            wo_sb = consts.tile([P, 2, D], bf16)

            if True:
                acts = consts
                xT = acts.tile([P, NDC, S], bf16)
                # pair-packed transposed projections: [2 heads x 64, S]
                qT = acts.tile([P, 2, S], bf16)
                kT = acts.tile([P, 2, S], bf16)
                # v augmented with a ones column (row 65 of the AV matmul
                # accumulates the softmax denominator): [s, j-tile, head, 65]
                va = acts.tile([P, NST, HPC, DK + 1], bf16)
                # Wo lhsT: [head-dim pair-chunk, pair, i]
                stack = acts.tile([P, 2, S], bf16)

                nc.scalar.dma_start(
                    wo_sb[:], wo.rearrange("p (c d) -> p c d", c=2)
                )

                def short(nm):
                    return ps.tile([P, 2 * NB], f32, tag="short", bufs=2, name=nm)

                def small(nm):
                    return ps.tile([P, NB], f32, tag="small", bufs=2, name=nm)

                def long_(nm):
                    # orientation-C AV accumulator: [q, head, q-tile, 68]
                    # (cols 0-63 dims, col 64 softmax denominator, 68 pads
                    # the per-block stride to a 16B multiple)
                    return ps.tile([P, 2, NB // P, P], f32, tag="long",
                                   bufs=1, name=nm)

                # ---------------- projections -------------------------------
                def emit_xdma(c, q4):
                    # x arrives host-pre-transposed [D, S]: plain contiguous
                    # 2D DMA per (chunk, seq-quarter) -> 32 cheap descriptors
                    nc.sync.dma_start(
                        xT[:, c, q4 * NB:(q4 + 1) * NB],
                        xbh[c * P:(c + 1) * P, q4 * NB:(q4 + 1) * NB],
                    )

                def emit_vproj(it):
                    _CUR_TAG[0] = f"vp{it}"
                    vp = small(f"vp{it}")
                    for d in range(NDC):
                        nc.tensor.matmul(
                            vp[:, 0:HD],
                            xT[:, d, it * P:(it + 1) * P],
                            wv_sb[:, d, :],
                            start=(d == 0),
                            stop=(d == NDC - 1),
                        )
                    nc.vector.tensor_copy(
                        out=va[:, it, :, 0:DK],
                        in_=vp[:, 0:HD].rearrange("p (h e) -> p h e", h=HPC),
                    )

                def emit_proj_part(sb, p, col, part):
                    # half of a q/k projection (4 d-chunks): small enough to
                    # fit a j-step's PE slack without stalling the exp stream
                    _CUR_TAG[0] = f"pj{sb}_{p}_{col}"
                    w_sb, b_sb, dT = (
                        (wq_sb, bq_sb, qT) if col == 0 else (wk_sb, bk_sb, kT)
                    )
                    if part == 0:
                        pq = small(f"pj{sb}_{p}_{col}")
                        state[("pj", sb, p, col)] = pq
                    else:
                        pq = state.pop(("pj", sb, p, col))
                    for d in range(4 * part, 4 * part + 4):
                        nc.tensor.matmul(
                            pq[:],
                            w_sb[:, d, p * P:(p + 1) * P],
                            xT[:, d, sb * NB:(sb + 1) * NB],
                            start=(d == 0),
                            stop=(d == NDC - 1),
                        )
                    if part == 1:
                        with nc.allow_low_precision("bf16 q/k feed scores"):
                            nc.vector.tensor_scalar_add(
                                out=dT[:, p, sb * NB:(sb + 1) * NB],
                                in0=pq[:],
                                scalar1=b_sb[:, p:p + 1],
                            )

                def emit_proj_half(sb, p, col):
                    emit_proj_part(sb, p, col, 0)
                    emit_proj_part(sb, p, col, 1)

                # ---------------- attention helpers -------------------------
                # all pair-0 units first; projections/finish work are drained
                # from a dependency-tracked background queue into step slots
                units = [(ib, 0) for ib in range(NSB)] + [
                    (ib, 1) for ib in range(NSB)
                ]

                def emit_scores(u, j):
                    _CUR_TAG[0] = f"sc{u}_{j}"
                    ib, p = units[u]
                    i0 = ib * NB
                    sc = short(f"sc{u}_{j}")
                    nc.tensor.matmul(
                        sc[:, 0:NB],
                        kT[0:DK, p, j * P:(j + 1) * P],
                        qT[0:DK, p, i0:i0 + NB],
                        tile_position=(0, 0),
                    )
                    nc.tensor.matmul(
                        sc[:, NB:2 * NB],
                        kT[DK:2 * DK, p, j * P:(j + 1) * P],
                        qT[DK:2 * DK, p, i0:i0 + NB],
                        tile_position=(64, 0),
                    )
                    return sc

                def emit_scores_pair(u, j):
                    _CUR_TAG[0] = f"scp{u}_{j}"
                    # boundary variant on the 1-bank ring: the 2-deep score
                    # ring drains to zero depth at unit boundaries (its slot
                    # frees only when exp(u-1, 14) completes), which stalls
                    # the exp stream ~2us; these bypass that ring
                    ib, p = units[u]
                    i0 = ib * NB
                    pair = []
                    for h in range(2):
                        sch = small(f"sc{u}_{j}_{h}")
                        nc.tensor.matmul(
                            sch[:, 0:NB],
                            kT[h * DK:(h + 1) * DK, p, j * P:(j + 1) * P],
                            qT[h * DK:(h + 1) * DK, p, i0:i0 + NB],
                            tile_position=(h * DK, 0),
                        )
                        pair.append(sch)
                    return tuple(pair)

                def emit_tc(u, g):
                    # transpose two normalized [128 q, 64] blocks of unit u
                    # into the Wo lhsT (stack) via tiny PE transposes
                    _CUR_TAG[0] = f"tc{u}_{g}"
                    ib, p = units[u]
                    i0 = ib * NB
                    ao = state[f"ao{u}"]
                    for h, qt in (divmod(2 * g, NSB), divmod(2 * g + 1, NSB)):
                        trp = small(f"tc{u}_{h}_{qt}")
                        nc.tensor.matmul(
                            trp[0:DK, 0:P], ao[:, h, qt, :], identb[:],
                        )
                        with nc.allow_low_precision("bf16 stack feeds Wo"):
                            nc.vector.tensor_copy(
                                out=stack[
                                    h * DK:(h + 1) * DK, p,
                                    i0 + qt * P:i0 + (qt + 1) * P,
                                ],
                                in_=trp[0:DK, 0:P],
                            )
                    if g == 3:
                        del state[f"ao{u}"]

                def emit_fin(ib, t, tail=False):
                    _CUR_TAG[0] = f"fin{ib}_{t}"
                    # output projection for row-tile t of i-block ib, split
                    # into two D-halves on the 1-bank "small" ring; at the
                    # tail the idle score ring is used for every other half
                    # so two halves overlap
                    it = ib * (NB // P) + t
                    for nbi in range(2):
                        fint = (
                            short(f"fin{it}_{nbi}") if (tail and nbi == 1)
                            else small(f"fin{it}_{nbi}")
                        )
                        fin = fint[:, 0:NB]
                        for pch in range(2):
                            nc.tensor.matmul(
                                fin,
                                stack[:, pch, it * P:(it + 1) * P],
                                wo_sb[:, pch, nbi * NB:(nbi + 1) * NB],
                                start=(pch == 0),
                                stop=(pch == 1),
                            )
                        ot = acts.tile(
                            [P, NB], f32, tag="ot", bufs=3, name=f"ot{it}_{nbi}"
                        )
                        nc.vector.tensor_copy(out=ot[:], in_=fin)
                        # after the exp stream ends the scalar HWDGE queue is
                        # idle; mid-stream it must not carry DMAs (each DMA
                        # occupies the ACT sequencer ~600ns and stalls exp)
                        eng = nc.scalar if (tail and nbi == 1) else nc.sync
                        eng.dma_start(
                            outp[it * P:(it + 1) * P, nbi * NB:(nbi + 1) * NB],
                            ot[:],
                        )

                from collections import deque

                sc_q = deque()
                fin_q = deque()
                sch_steps = {
                    (u, j) for u in range(1, 8) for j in (5, 7, 9, 11, 13)
                }
                state = {"pending": None, "po": None}
                work_q = deque()
                done = set()
                # virtual timeline: drain background PE work only while the
                # PE is ahead of the ACT exp cadence; co-running PE work under
                # the exp stream costs ACT ~20% (SBUF/PSUM port contention),
                # so the queue must neither starve scores nor front-load
                budget = {"pe": 0.0, "act": 0.0, "j": 0}
                COST = {"vp": 950.0, "pj": 1800.0, "pjB": 900.0, "fin": 950.0, "tc": 250.0}

                def drain_one():
                    if not work_q:
                        return False
                    key, fn, args = work_q.popleft()
                    if key[0] == "pj":
                        emit_proj_part(*key[1:], 0)
                        work_q.appendleft((("pjB",) + key[1:],
                                           emit_proj_part, key[1:] + (1,)))
                        budget["pe"] += 900.0
                        return True
                    fn(*args)
                    done.add(key)
                    if key[0] == "pjB":
                        done.add(("pj",) + key[1:])
                    budget["pe"] += COST.get(key[0], 700.0)
                    return True

                def drain_budget():
                    # cap per-step drains: a burst of background work between
                    # two exps stalls the exp stream for its full duration
                    # (each exp's sem wait covers all prior PE work)
                    spent = 0.0
                    while work_q and budget["pe"] + 300.0 < budget["act"]:
                        c = COST.get(work_q[0][0][0], 700.0)
                        if spent > 0.0 and spent + c > 2800.0:
                            break
                        drain_one()
                        spent += c

                def ensure(*keys):
                    need = [k for k in keys if k not in done]
                    while need:
                        assert work_q, f"missing work for {need}"
                        drain_one()
                        need = [k for k in need if k not in done]

                def emit_unit_end(u):
                    # the ao ring is 2 deep: unit u's normalize reuses the
                    # slot of u-2, whose readers are the tc items queued at
                    # our j==3 -- they must be emitted before we are
                    if u >= 1:
                        ensure(*[("tc", u - 1, g) for g in range(4)])
                    # per-query reciprocal of the 8 denominator columns, then
                    # normalize each [128 q, 64] block into bf16 (DVE); the
                    # accumulator bank frees right here, not at the next bc
                    po = state["po"]
                    rcp = acts.tile([P, 16], f32, tag="rcp", bufs=2,
                                    name=f"rcp{u}")
                    with nc.allow_low_precision("approx recip of sumexp"):
                        for h in range(2):
                            for qt in range(NSB):
                                i_ = 4 * h + qt
                                nc.vector.reciprocal(
                                    out=rcp[:, i_:i_ + 1],
                                    in_=po[:, h, qt, 64:65],
                                )
                    ao = acts.tile([P, 2, NSB, DK], bf16, tag="aos", bufs=2,
                                   name=f"ao{u}")
                    with nc.allow_low_precision("bf16 normalized attn out"):
                        for h in range(2):
                            for qt in range(NSB):
                                nc.vector.tensor_scalar_mul(
                                    out=ao[:, h, qt, :],
                                    in0=po[:, h, qt, 0:DK],
                                    scalar1=rcp[:, 4 * h + qt:4 * h + qt + 1],
                                )
                    state[f"ao{u}"] = ao
                    state["pending"] = u

                def emit_step(u, j):
                    # one attention j-step of unit u
                    ib, p = units[u]
                    if j == 0:
                        state["po"] = long_(f"po{u}")
                        # PE start=True wipes the accumulation state of the
                        # whole PSUM bank (not just the written region), so
                        # zero the 8 sub-blocks here and accumulate with
                        # start=False throughout
                        nc.vector.memset(state["po"][:, :, :, 0:65], 0.0)
                    po = state["po"]
                    sc = sc_q.popleft()
                    ex = acts.tile(
                        [P, 2 * NB], bf16, tag="ex", bufs=6, name=f"ex{u}_{j}"
                    )
                    if isinstance(sc, tuple):
                        for h in range(2):
                            nc.scalar.activation(
                                ex[:, h * NB:(h + 1) * NB], sc[h][:, 0:NB],
                                Exp, scale=0.125,
                            )
                    elif (u, j) in sch_steps:
                        # Schraudolph exp on the DVE, concurrent with ACT's
                        # exp stream: bf16 bits of int16(a*s + b) approximate
                        # exp(s*0.125) (a = 16/ln2, sawtooth rms ~1.8%, mean
                        # bias ~0 and cancels in softmax anyway)
                        with nc.allow_low_precision("schraudolph exp"):
                            nc.vector.tensor_scalar(
                                out=ex.bitcast(mybir.dt.int16)[:],
                                in0=sc[:],
                                scalar1=23.083120654223414,
                                scalar2=16248.0,
                                op0=mybir.AluOpType.mult,
                                op1=mybir.AluOpType.add,
                            )
                    else:
                        nc.scalar.activation(ex[:], sc[:], Exp, scale=0.125)
                    nj = j + 2
                    if nj < NST:
                        ensure(("pj", ib, p, 0), ("pj", nj // 4, p, 1))
                        sc_q.append(emit_scores(u, nj))
                    elif u + 1 < 8:
                        ib2, p2 = units[u + 1]
                        ensure(("pj", ib2, p2, 0), ("pj", 0, p2, 1))
                        if work_q and work_q[0][0][0] == "pjB":
                            drain_one()
                        sc_q.append(emit_scores_pair(u + 1, nj - NST))
                    def emit_av(aj, aex):
                        _CUR_TAG[0] = f"av{u}_{aj}"
                        ensure(("vp", aj))
                        for h in range(2):
                            for qt in range(NSB):
                                nc.tensor.matmul(
                                    po[:, h, qt, 0:DK + 1],
                                    aex[:, h * NB + qt * P:
                                        h * NB + (qt + 1) * P],
                                    va[:, aj, 2 * p + h, :],
                                    start=False,
                                    stop=(aj == NST - 1),
                                    skip_group_check=True,
                                )

                    # defer the first two AVs to step 2: the po-ring WAR
                    # (drains of the previous unit on DVE) then never heads
                    # the in-order PE queue at unit boundaries
                    if j in (0, 1):
                        state.setdefault("av_defer", []).append((j, ex))
                    else:
                        for a in state.pop("av_defer", []):
                            emit_av(*a)
                        emit_av(j, ex)
                    # previous-unit finish + budget-paced background work
                    budget["act"] += 1300.0
                    budget["pe"] += 680.0
                    if state["pending"] is not None and j == 3:
                        pu = state["pending"]
                        for g in range(4):
                            work_q.append((("tc", pu, g), emit_tc, (pu, g)))
                        if units[pu][1] == 1:
                            for t in range(NB // P):
                                work_q.append((
                                    ("fin", units[pu][0], t),
                                    emit_fin, (units[pu][0], t),
                                ))
                        state["pending"] = None
                    drain_budget()
                    if j == NST - 1:
                        emit_unit_end(u)

                # ---------------- fused emission ----------------------------
                # x chunk DMAs up front (quarter-major so the first seq block
                # lands ~2.8us in); then block-0 q/k projections + first two
                # score tiles, so the exp stream starts ~5us into the kernel.
                # Everything else drains from work_q one slot per j-step.
                for q4 in range(NSB):
                    for c in range(NDC):
                        emit_xdma(c, q4)
                nc.vector.memset(
                    va[:, :, :, DK:DK + 1].bitcast(mybir.dt.uint16), 0x3F80
                )
                _CUR_TAG[0] = "pj0_0_x"
                pqk = small("pj0_0_1")
                pqq = small("pj0_0_0")
                for d in range(NDC):
                    nc.tensor.matmul(
                        pqk[:], wk_sb[:, d, 0:P], xT[:, d, 0:NB],
                        start=(d == 0), stop=(d == NDC - 1),
                    )
                    nc.tensor.matmul(
                        pqq[:], wq_sb[:, d, 0:P], xT[:, d, 0:NB],
                        start=(d == 0), stop=(d == NDC - 1),
                    )
                with nc.allow_low_precision("bf16 q/k feed scores"):
                    nc.vector.tensor_scalar_add(
                        out=kT[:, 0, 0:NB], in0=pqk[:], scalar1=bk_sb[:, 0:1]
                    )
                    nc.vector.tensor_scalar_add(
                        out=qT[:, 0, 0:NB], in0=pqq[:], scalar1=bq_sb[:, 0:1]
                    )
                done.add(("pj", 0, 0, 1))
                done.add(("pj", 0, 0, 0))
                sc_q.append(emit_scores(0, 0))
                sc_q.append(emit_scores(0, 1))
                wk_order = (
                    [("vp", 0), ("pj", 1, 0, 1), ("vp", 1), ("pj", 2, 0, 1),
                     ("vp", 2), ("pj", 3, 0, 1)]
                    + [("vp", it) for it in range(3, NST)]
                    + [("pj", sb, 0, 0) for sb in range(1, NSB)]
                    + [("pj", sb, 1, c) for sb in range(NSB) for c in (1, 0)]
                )
                for key in wk_order:
                    if key[0] == "vp":
                        work_q.append((key, emit_vproj, key[1:]))
                    else:
                        work_q.append((key, emit_proj_half, key[1:]))
                for u in range(8):
                    for j in range(NST):
                        emit_step(u, j)
                # tail: finish of the last unit + any remaining queued work
                while work_q:
                    drain_one()
                pu = state["pending"]
                for g in range(4):
                    emit_tc(pu, g)
                for t in range(NB // P):
                    emit_fin(units[pu][0], t, tail=True)

    _split_excess_waits(nc)
    return nc


def _get_program():
    if "nc" not in _CACHE:
        _CACHE["nc"] = _build_program()
    return _CACHE["nc"]


def _prep_w(w):
    # [n*128, m] -> [128, n*m]: partition p holds rows {c*128+p}, contiguous
    n = w.shape[0] // P
    return np.ascontiguousarray(
        w.reshape(n, P, w.shape[1]).transpose(1, 0, 2).reshape(P, -1)
    )


def kernel(x, Wq, bq, Wk, bk, Wv, bv, Wo, bo, _trace=False):
    import ml_dtypes
    from concourse.bass_utils import run_bass_kernel_spmd

    bft = np.dtype(ml_dtypes.bfloat16)
    x = np.asarray(x, dtype=np.float32)
    Wq = np.asarray(Wq, dtype=np.float32)
    Wk = np.asarray(Wk, dtype=np.float32)
    Wv = np.asarray(Wv, dtype=np.float32)
    Wo = np.asarray(Wo, dtype=np.float32)
    bq = np.asarray(bq, dtype=np.float32)
    bk = np.asarray(bk, dtype=np.float32)
    bv = np.asarray(bv, dtype=np.float32)
    bo = np.asarray(bo, dtype=np.float32)

    in_maps = []
    for c in range(NCORES):
        b = c // 4
        cs = (c % 4) * HD
        in_maps.append({
            "xbh": np.ascontiguousarray(x[b].astype(bft).T),
            "wq": _prep_w(Wq[:, cs:cs + HD].astype(bft)),
            "wk": _prep_w(Wk[:, cs:cs + HD].astype(bft)),
            "wv": _prep_w(Wv[:, cs:cs + HD].astype(bft)),
            "wo": _prep_w(Wo[cs:cs + HD, :].astype(bft)),
            "bqt": np.ascontiguousarray(bq[cs:cs + HD].reshape(2, P).T),
            "bkt": np.ascontiguousarray(bk[cs:cs + HD].reshape(2, P).T),
        })

    nc = _get_program()
    res = run_bass_kernel_spmd(
        nc, in_maps, core_ids=list(range(NCORES)), trace=_trace
    )

    cvec = (bv @ Wo + bo).astype(np.float32)
    out = np.empty((B, S, D), dtype=np.float32)
    for b in range(B):
        acc = res.results[4 * b]["outp"].astype(np.float64)
        for c in range(4 * b + 1, 4 * b + 4):
            acc = acc + res.results[c]["outp"]
        out[b] = (acc + cvec).astype(np.float32)

    if _trace:
        _CACHE["last_results"] = res
    return out

